# revision 44
# baseline (speedup 1.0000x reference)
"""Trainium2 Bass kernel for ContextHyperLinearSSM.

Computes out[b,:] = x[b,:] @ (WA[context[b]] * adj_xx) + u[b,:] @ (WB[context[b]] * adj_xu)

Strategy: shard the CONTEXT axis across the 8 cores (64 contexts each).
The host groups samples by context (padded to the max group size G), so each
core streams its 64 contexts' weight banks from HBM exactly once, applies the
adjacency masks on-device, and runs 3 accumulating matmuls per context
(two K=128 chunks of the A term + one for the B term).  Each sample's row is
computed by exactly one core, so the host-side unshard is a pure scatter.
"""

import sys

sys.path.insert(0, "/opt/trn_rl_repo")

import numpy as np

import concourse.bass as bass
import concourse.mybir as mybir
import concourse.tile as tile
from concourse import bacc
from concourse.bass import ts
from concourse.bass_utils import run_bass_kernel_spmd

N_CORES = 8
CT = 8  # contexts per DMA group

# matmul operand dtype: float32 (4 cyc/row) or float32r (1 cyc/row at N>=256)
MM_DT = mybir.dt.float32


def _install_profile_shim():
    """Register the NTFF profile hook that trn_boot skips when
    antenv.axon_hooks is missing from the image (profiling only)."""
    import types
    if "antenv.axon_hooks" in sys.modules:
        return
    try:
        from trn_agent_boot.trn_boot import _ntff_profile_via_ctypes
        hook = _ntff_profile_via_ctypes("/opt/axon/libaxon_pjrt.so")
    except Exception:
        hook = None
    mod = types.ModuleType("antenv.axon_hooks")
    mod.get_axon_ntff_profile_hook = lambda: hook
    mod.set_axon_ntff_profile_hook = lambda h: None
    sys.modules["antenv.axon_hooks"] = mod


def _build_program(CP, S, A, G):
    """Build the per-core Bass program. CP contexts/core, group size G."""
    f32 = mybir.dt.float32
    nc = bacc.Bacc("TRN2", target_bir_lowering=False)

    HS = S // 128  # number of 128-row K-chunks of the A-term contraction
    assert S % 128 == 0 and A == 128
    NG = CP // CT
    assert CP % CT == 0
    # PSUM context packing: FF contexts along the free dim of a bank and
    # (for G <= 64) two partition slots at 0/64 -- matmul outputs may only
    # base at partition 0/32/64.  CPT contexts share one bank; T banks/group.
    FF = max(1, min(CT, 512 // S))
    PSL = 2 if G <= 64 else 1
    CPT = min(CT, PSL * FF)
    T = -(-CT // CPT)
    assert T * CPT == CT, (CT, FF, PSL, CPT)

    # weight/activation blobs are pre-laid-out by the host so every group
    # DMA is one fully contiguous span with 128-partition lines
    wa = nc.dram_tensor("wa", [NG, 128, CT, HS, S], f32,
                        kind="ExternalInput").ap()
    wb = nc.dram_tensor("wb", [NG, 128, CT, S], f32, kind="ExternalInput").ap()
    xt = nc.dram_tensor("xt", [NG, 128, CT, HS, G], f32,
                        kind="ExternalInput").ap()
    ut = nc.dram_tensor("ut", [NG, 128, CT, G], f32, kind="ExternalInput").ap()
    adj_xx = nc.dram_tensor("adj_xx", [HS, 128, S], mybir.dt.uint8,
                            kind="ExternalInput").ap()
    adj_xu = nc.dram_tensor("adj_xu", [A, S], mybir.dt.uint8,
                            kind="ExternalInput").ap()
    # output blob: [group][partition-slot][sample][bank][context-half][s]
    out = nc.dram_tensor("out", [NG, PSL, G, T, FF, S], f32,
                         kind="ExternalOutput").ap()

    with tile.TileContext(nc) as tc:
        with (
            tc.tile_pool(name="const", bufs=1) as const,
            tc.tile_pool(name="w", bufs=3) as wpool,
            tc.tile_pool(name="xu", bufs=3) as xpool,
            tc.tile_pool(name="o", bufs=3) as opool,
            tc.tile_pool(name="psum", bufs=8, space="PSUM") as psum,
        ):
            # adjacency masks, cast uint8 -> f32 during the (SWDGE) DMA
            adjA0 = const.tile([128, HS, S], f32)
            nc.gpsimd.dma_start(adjA0[:], adj_xx.rearrange("h p s -> p h s"))
            adjB0 = const.tile([128, S], f32)
            nc.gpsimd.dma_start(adjB0[:], adj_xu[:])
            # funnel both masks through DVE so the per-group mask
            # multiplies carry same-engine deps only
            adjA = const.tile([128, HS, S], f32)
            nc.vector.tensor_copy(adjA[:], adjA0[:])
            adjB = const.tile([128, S], f32)
            nc.vector.tensor_copy(adjB[:], adjB0[:])

            rounded = MM_DT == mybir.dt.float32r
            CH = CT // 2
            for g in range(NG):
                cs = ts(g, CT)
                # B-weights and activations first (they unblock the PE's
                # leading matmul of every context), then A-weights in halves
                wb_t = wpool.tile([128, CT, S], f32)
                nc.sync.dma_start(wb_t[:], wb[g])
                xt_t = xpool.tile([128, CT, HS, G], f32)
                nc.sync.dma_start(xt_t[:], xt[g])
                ut_t = xpool.tile([128, CT, G], f32)
                nc.sync.dma_start(ut_t[:], ut[g])
                wa_t = wpool.tile([128, CT, HS, S], f32)
                for half in range(2):
                    hs = slice(half * CH, (half + 1) * CH)
                    nc.sync.dma_start(wa_t[:, hs], wa[g][:, hs])

                if rounded:
                    # fp32r consumers need fp32r-rounded producers
                    wa_m = wpool.tile([128, CT, HS, S], MM_DT, tag="wa_m")
                    wb_m = wpool.tile([128, CT, S], MM_DT, tag="wb_m")
                    xt_m = xpool.tile([128, CT, HS, G], MM_DT, tag="xt_m")
                    ut_m = xpool.tile([128, CT, G], MM_DT, tag="ut_m")
                    nc.vector.tensor_copy(xt_m[:], xt_t[:])
                    nc.vector.tensor_copy(ut_m[:], ut_t[:])
                else:
                    wa_m, wb_m, xt_m, ut_m = wa_t, wb_t, xt_t, ut_t

                # mask the weights on DVE in halves, B before A within each
                # half (B gates each context's leading matmul)
                adjA_b = adjA[:].rearrange("p h s -> p (h s)")[:, None, :] \
                    .to_broadcast([128, CH, HS * S])
                adjB_b = adjB[:, None, :].to_broadcast([128, CH, S])
                for half in range(2):
                    hs = slice(half * CH, (half + 1) * CH)
                    nc.vector.tensor_tensor(
                        wb_m[:, hs], wb_t[:, hs], adjB_b,
                        mybir.AluOpType.mult)
                    nc.vector.tensor_tensor(
                        wa_m[:, hs].rearrange("p c h s -> p c (h s)"),
                        wa_t[:, hs].rearrange("p c h s -> p c (h s)"),
                        adjA_b, mybir.AluOpType.mult)

                # x-stationary matmuls streaming masked weights (N=S rows);
                # all CT contexts of a group pack into ONE psum bank:
                # context c -> partition slot c//FF, free half c%FF
                ps_tiles = [psum.tile([128, FF * S], f32, tag="ps",
                                      name=f"ps_{g}_{t}")
                            for t in range(T)]
                for c in range(CT):
                    t, r2 = divmod(c, CPT)
                    sl, cf = divmod(r2, FF)
                    pslice = ps_tiles[t][sl * 64:sl * 64 + G,
                                         cf * S:cf * S + S]
                    nc.tensor.matmul(
                        pslice,
                        lhsT=ut_m[:, c, :],
                        rhs=wb_m[:, c, :],
                        start=True, stop=False)
                    for h in range(HS):
                        nc.tensor.matmul(
                            pslice,
                            lhsT=xt_m[:, c, h, :],
                            rhs=wa_m[:, c, h, :],
                            start=False, stop=(h == HS - 1))
                out_sb = opool.tile([128, T, FF, S], f32)
                for t in range(T):
                    nc.scalar.copy(
                        out_sb[:, t].rearrange("p f s -> p (f s)"),
                        ps_tiles[t][:])
                for sl in range(PSL):
                    nc.scalar.dma_start(
                        out[g, sl], out_sb[sl * 64:sl * 64 + G])

    nc.compile()
    return nc


def kernel(x, u, WA, WB, adj_xx, adj_xu, context, _trace=False):
    B, S = x.shape
    _, A = u.shape
    C = WA.shape[0]
    assert C % N_CORES == 0
    CP = C // N_CORES

    # ---- host-side shard: group samples by context --------------------
    context = np.asarray(context)
    cnt = np.bincount(context, minlength=C)
    G = int(cnt.max())
    G = max(4, ((G + 3) // 4) * 4)
    order = np.argsort(context, kind="stable")
    starts = np.zeros(C + 1, np.int64)
    starts[1:] = np.cumsum(cnt)
    j = np.arange(G)
    valid = j[None, :] < cnt[:, None]                      # [C, G]
    pos = starts[:-1, None] + np.minimum(j[None, :],
                                         np.maximum(cnt[:, None] - 1, 0))
    gidx = order[pos]                                      # [C, G]

    Xp = np.asarray(x, np.float32)[gidx]                   # [C, G, S]
    Up = np.asarray(u, np.float32)[gidx]                   # [C, G, A]
    XpT = np.ascontiguousarray(Xp.transpose(0, 2, 1))      # [C, S, G]
    UpT = np.ascontiguousarray(Up.transpose(0, 2, 1))      # [C, A, G]

    WA = np.ascontiguousarray(WA, np.float32)
    WB = np.ascontiguousarray(WB, np.float32)
    adjxx_u8 = np.ascontiguousarray(adj_xx).view(np.uint8).reshape(S // 128, 128, S)
    adjxu_u8 = np.ascontiguousarray(adj_xu).view(np.uint8)

    HS = S // 128
    NG = CP // CT
    in_maps = []
    for k in range(N_CORES):
        sl = slice(k * CP, (k + 1) * CP)
        # relayout: group DMAs become contiguous [128, CT*HS*S] spans
        wa_k = np.ascontiguousarray(
            WA[sl].reshape(NG, CT, HS, 128, S).transpose(0, 3, 1, 2, 4))
        wb_k = np.ascontiguousarray(
            WB[sl].reshape(NG, CT, 128, S).transpose(0, 2, 1, 3))
        xt_k = np.ascontiguousarray(
            XpT[sl].reshape(NG, CT, HS, 128, G).transpose(0, 3, 1, 2, 4))
        ut_k = np.ascontiguousarray(
            UpT[sl].reshape(NG, CT, 128, G).transpose(0, 2, 1, 3))
        in_maps.append({
            "wa": wa_k,
            "wb": wb_k,
            "xt": xt_k,
            "ut": ut_k,
            "adj_xx": adjxx_u8,
            "adj_xu": adjxu_u8,
        })

    if _trace:
        _install_profile_shim()
    nc = _build_program(CP, S, A, G)
    res = run_bass_kernel_spmd(nc, in_maps, core_ids=list(range(N_CORES)),
                               trace=_trace)

    # device output blobs [NG, PSL, G, T, FF, S] -> [CP, G, S].
    # context c in a group lives at bank t=c//CPT, partition slot
    # sl=(c%CPT)//FF (64-aligned), free half cf=c%FF.
    outs = []
    for r in res.results:
        v = r["out"]
        # axes (g, sl, gg, t, cf, s) -> (g, t, sl, cf, gg, s)
        v = v.transpose(0, 3, 1, 4, 2, 5).reshape(CP, G, S)
        outs.append(v)
    Out_all = np.concatenate(outs, axis=0)                 # [C, G, S]
    out_full = np.zeros((B, S), np.float32)
    out_full[gidx[valid]] = Out_all[valid]

    if _trace:
        return out_full, res
    return out_full


# revision 48
# speedup vs baseline: 1.0486x; 1.0486x over previous
"""Trainium2 Bass kernel for ContextHyperLinearSSM.

Computes out[b,:] = x[b,:] @ (WA[context[b]] * adj_xx) + u[b,:] @ (WB[context[b]] * adj_xu)

Strategy: shard the CONTEXT axis across the 8 cores (64 contexts each).
The host groups samples by context (padded to the max group size G), so each
core streams its 64 contexts' weight banks from HBM exactly once, applies the
adjacency masks on-device, and runs 3 accumulating matmuls per context.
Each sample's row is computed by exactly one core, so the host-side unshard
is a pure scatter.

Device-side layout: contexts are processed in groups of CT; each half-group's
payload (B-weights, A-weights, x/u activations) is packed by the host into a
single contiguous HBM blob so one DMA per half-group runs at full descriptor
efficiency.  A single in-place DVE multiply against a combined [adjB|adjA]
mask tile masks a half-group's weights.  All CT contexts of a group accumulate
into one PSUM bank (two 64-aligned partition slots x two free halves), so one
ACT copy per bank drains PSUM.
"""

import sys

sys.path.insert(0, "/opt/trn_rl_repo")

import numpy as np

import concourse.bass as bass
import concourse.mybir as mybir
import concourse.tile as tile
from concourse import bacc
from concourse.bass_utils import run_bass_kernel_spmd

N_CORES = 8
CT = 8  # contexts per PSUM group
W_BUFS = 4

# matmul operand dtype: float32 (exact) or float32r (tf32-like, 4x PE rate)
MM_DT = mybir.dt.float32


def _install_profile_shim():
    """Register the NTFF profile hook that trn_boot skips when
    antenv.axon_hooks is missing from the image (profiling only)."""
    import types
    if "antenv.axon_hooks" in sys.modules:
        return
    try:
        from trn_agent_boot.trn_boot import _ntff_profile_via_ctypes
        hook = _ntff_profile_via_ctypes("/opt/axon/libaxon_pjrt.so")
    except Exception:
        hook = None
    mod = types.ModuleType("antenv.axon_hooks")
    mod.get_axon_ntff_profile_hook = lambda: hook
    mod.set_axon_ntff_profile_hook = lambda h: None
    sys.modules["antenv.axon_hooks"] = mod


def _build_program(CP, S, A, G):
    """Build the per-core Bass program. CP contexts/core, group size G."""
    f32 = mybir.dt.float32
    nc = bacc.Bacc("TRN2", target_bir_lowering=False)

    HS = S // 128  # 128-row K-chunks of the A contraction
    K = HS + 1     # matmuls per context (1 B-term + HS A-terms)
    assert S % 128 == 0 and A == 128
    NG = CP // CT
    CH = CT // 2   # contexts per half-group payload
    assert CP % CT == 0 and CT % 2 == 0
    WF = CH * K * S   # weight f32 per partition line per half-group
    AF = CH * K * G   # activation f32 per partition line per half-group

    # PSUM packing: FF contexts along the free dim of a bank, two 64-aligned
    # partition slots (matmul out base partition must be 0/32/64)
    FF = max(1, min(CT, 512 // S))
    PSL = 2 if G <= 64 else 1
    CPT = min(CT, PSL * FF)
    T = -(-CT // CPT)
    assert T * CPT == CT, (CT, FF, PSL, CPT)

    blob = nc.dram_tensor("blob", [NG, 2, 128, WF + AF], f32,
                          kind="ExternalInput").ap()
    adj_xx = nc.dram_tensor("adj_xx", [HS, 128, S], mybir.dt.uint8,
                            kind="ExternalInput").ap()
    adj_xu = nc.dram_tensor("adj_xu", [A, S], mybir.dt.uint8,
                            kind="ExternalInput").ap()
    # output blob: [group][partition-slot][sample][bank][context-half][s]
    out = nc.dram_tensor("out", [NG, PSL, G, T, FF, S], f32,
                         kind="ExternalOutput").ap()

    rounded = MM_DT == mybir.dt.float32r

    with tile.TileContext(nc) as tc:
        with (
            tc.tile_pool(name="const", bufs=1) as const,
            tc.tile_pool(name="w", bufs=W_BUFS) as wpool,
            tc.tile_pool(name="o", bufs=3) as opool,
            tc.tile_pool(name="psum", bufs=8, space="PSUM") as psum,
        ):
            # combined [adjB | adjA] mask, cast u8->f32 during the SWDGE DMA,
            # then funneled through DVE (same-engine dep for the mask TTs)
            adjB0 = const.tile([128, S], f32)
            nc.gpsimd.dma_start(adjB0[:], adj_xu[:])
            adjA0 = const.tile([128, HS, S], f32)
            nc.gpsimd.dma_start(adjA0[:], adj_xx.rearrange("h p s -> p h s"))
            adjC = const.tile([128, K * S], f32)
            nc.vector.tensor_copy(adjC[:, :S], adjB0[:])
            nc.vector.tensor_copy(
                adjC[:, S:], adjA0[:].rearrange("p h s -> p (h s)"))
            adjC_b = adjC[:, None, :].to_broadcast([128, CH, K * S])

            for g in range(NG):
                halves = []
                for hf in range(2):
                    hb = wpool.tile([128, WF + AF], f32, tag="hb",
                                    name=f"hb_{g}_{hf}")
                    nc.sync.dma_start(hb[:], blob[g, hf])
                    wv = hb[:, :WF].rearrange("p (c k s) -> p c k s",
                                              c=CH, k=K)
                    av = hb[:, WF:].rearrange("p (c k g) -> p c k g",
                                              c=CH, k=K)
                    if rounded:
                        wm = wpool.tile([128, WF], MM_DT, tag="wm",
                                        name=f"wm_{g}_{hf}")
                        am = wpool.tile([128, AF], MM_DT, tag="am",
                                        name=f"am_{g}_{hf}")
                        nc.vector.tensor_copy(am[:], hb[:, WF:])
                        nc.vector.tensor_tensor(
                            wm[:].rearrange("p (c ks) -> p c ks", c=CH),
                            hb[:, :WF].rearrange("p (c ks) -> p c ks", c=CH),
                            adjC_b, mybir.AluOpType.mult)
                        wv = wm[:].rearrange("p (c k s) -> p c k s",
                                             c=CH, k=K)
                        av = am[:].rearrange("p (c k g) -> p c k g",
                                             c=CH, k=K)
                    else:
                        # mask B+A weights with ONE in-place multiply
                        nc.vector.tensor_tensor(
                            hb[:, :WF].rearrange("p (c ks) -> p c ks", c=CH),
                            hb[:, :WF].rearrange("p (c ks) -> p c ks", c=CH),
                            adjC_b, mybir.AluOpType.mult)
                    halves.append((wv, av))

                ps_tiles = [psum.tile([128, FF * S], f32, tag="ps",
                                      name=f"ps_{g}_{t}")
                            for t in range(T)]
                for c in range(CT):
                    hf, ci = divmod(c, CH)
                    wv, av = halves[hf]
                    t, r2 = divmod(c, CPT)
                    sl, cf = divmod(r2, FF)
                    pslice = ps_tiles[t][sl * 64:sl * 64 + G,
                                         cf * S:cf * S + S]
                    for k in range(K):
                        nc.tensor.matmul(
                            pslice,
                            lhsT=av[:, ci, k, :],
                            rhs=wv[:, ci, k, :],
                            start=(k == 0), stop=(k == K - 1))
                out_sb = opool.tile([128, T, FF, S], f32)
                for t in range(T):
                    for sl in range(PSL):
                        nc.scalar.copy(
                            out_sb[sl * 64:sl * 64 + G, t].rearrange(
                                "p f s -> p (f s)"),
                            ps_tiles[t][sl * 64:sl * 64 + G, :])
                for sl in range(PSL):
                    nc.scalar.dma_start(
                        out[g, sl], out_sb[sl * 64:sl * 64 + G])

    nc.compile()
    return nc


def kernel(x, u, WA, WB, adj_xx, adj_xu, context, _trace=False):
    B, S = x.shape
    _, A = u.shape
    C = WA.shape[0]
    assert C % N_CORES == 0
    CP = C // N_CORES
    HS = S // 128
    K = HS + 1
    NG = CP // CT
    CH = CT // 2

    # ---- host-side shard: group samples by context --------------------
    context = np.asarray(context)
    cnt = np.bincount(context, minlength=C)
    G = int(cnt.max())
    G = max(4, ((G + 3) // 4) * 4)
    order = np.argsort(context, kind="stable")
    starts = np.zeros(C + 1, np.int64)
    starts[1:] = np.cumsum(cnt)
    j = np.arange(G)
    valid = j[None, :] < cnt[:, None]                      # [C, G]
    pos = starts[:-1, None] + np.minimum(j[None, :],
                                         np.maximum(cnt[:, None] - 1, 0))
    gidx = order[pos]                                      # [C, G]

    Xp = np.asarray(x, np.float32)[gidx]                   # [C, G, S]
    Up = np.asarray(u, np.float32)[gidx]                   # [C, G, A]
    XpT = np.ascontiguousarray(Xp.transpose(0, 2, 1))      # [C, S, G]
    UpT = np.ascontiguousarray(Up.transpose(0, 2, 1))      # [C, A, G]

    WA = np.ascontiguousarray(WA, np.float32)
    WB = np.ascontiguousarray(WB, np.float32)
    adjxx_u8 = np.ascontiguousarray(adj_xx).view(np.uint8).reshape(HS, 128, S)
    adjxu_u8 = np.ascontiguousarray(adj_xu).view(np.uint8)

    WF = CH * K * S
    AF = CH * K * G
    in_maps = []
    for k in range(N_CORES):
        sl = slice(k * CP, (k + 1) * CP)
        # pack each half-group's weights+activations into one contiguous
        # blob: per partition line [CH, K, S] weights then [CH, K, G] acts,
        # slot 0 = B-term, slot 1+h = A-term K-chunk h
        W3 = np.empty((NG, 2, 128, CH, K, S), np.float32)
        W3[..., 0, :] = WB[sl].reshape(NG, 2, CH, 128, S) \
            .transpose(0, 1, 3, 2, 4)
        W3[..., 1:, :] = WA[sl].reshape(NG, 2, CH, HS, 128, S) \
            .transpose(0, 1, 4, 2, 3, 5)
        A3 = np.empty((NG, 2, 128, CH, K, G), np.float32)
        A3[..., 0, :] = UpT[sl].reshape(NG, 2, CH, 128, G) \
            .transpose(0, 1, 3, 2, 4)
        A3[..., 1:, :] = XpT[sl].reshape(NG, 2, CH, HS, 128, G) \
            .transpose(0, 1, 4, 2, 3, 5)
        blob = np.concatenate(
            [W3.reshape(NG, 2, 128, WF), A3.reshape(NG, 2, 128, AF)],
            axis=-1)
        in_maps.append({
            "blob": np.ascontiguousarray(blob),
            "adj_xx": adjxx_u8,
            "adj_xu": adjxu_u8,
        })

    if _trace:
        _install_profile_shim()
    nc = _build_program(CP, S, A, G)
    res = run_bass_kernel_spmd(nc, in_maps, core_ids=list(range(N_CORES)),
                               trace=_trace)

    # device output blobs [NG, PSL, G, T, FF, S] -> [CP, G, S].
    # context c in a group lives at bank t=c//CPT, partition slot
    # sl=(c%CPT)//FF (64-aligned), free half cf=c%FF.
    outs = []
    for r in res.results:
        v = r["out"]
        # axes (g, sl, gg, t, cf, s) -> (g, t, sl, cf, gg, s)
        v = v.transpose(0, 3, 1, 4, 2, 5).reshape(CP, G, S)
        outs.append(v)
    Out_all = np.concatenate(outs, axis=0)                 # [C, G, S]
    out_full = np.zeros((B, S), np.float32)
    out_full[gidx[valid]] = Out_all[valid]

    if _trace:
        return out_full, res
    return out_full


# revision 49
# speedup vs baseline: 1.1518x; 1.0985x over previous
"""Trainium2 Bass kernel for ContextHyperLinearSSM.

Computes out[b,:] = x[b,:] @ (WA[context[b]] * adj_xx) + u[b,:] @ (WB[context[b]] * adj_xu)

Strategy: shard the CONTEXT axis across the 8 cores (64 contexts each).
The host groups samples by context (padded to the max group size G), so each
core streams its 64 contexts' weight banks from HBM exactly once, applies the
adjacency masks on-device, and runs 3 accumulating matmuls per context.
Each sample's row is computed by exactly one core, so the host-side unshard
is a pure scatter.

Device-side layout: contexts are processed in groups of CT; each half-group's
payload (B-weights, A-weights, x/u activations) is packed by the host into a
single contiguous HBM blob so one DMA per half-group runs at full descriptor
efficiency.  A single in-place DVE multiply against a combined [adjB|adjA]
mask tile masks a half-group's weights.  All CT contexts of a group accumulate
into one PSUM bank (two 64-aligned partition slots x two free halves), so one
ACT copy per bank drains PSUM.
"""

import sys

sys.path.insert(0, "/opt/trn_rl_repo")

import numpy as np

import concourse.bass as bass
import concourse.mybir as mybir
import concourse.tile as tile
from concourse import bacc
from concourse.bass_utils import run_bass_kernel_spmd

N_CORES = 8
CT = 8  # contexts per PSUM group
W_BUFS = 4

# matmul operand dtype: float32 (exact) or float32r (tf32-like, 4x PE rate)
MM_DT = mybir.dt.float32


def _install_profile_shim():
    """Register the NTFF profile hook that trn_boot skips when
    antenv.axon_hooks is missing from the image (profiling only)."""
    import types
    if "antenv.axon_hooks" in sys.modules:
        return
    try:
        from trn_agent_boot.trn_boot import _ntff_profile_via_ctypes
        hook = _ntff_profile_via_ctypes("/opt/axon/libaxon_pjrt.so")
    except Exception:
        hook = None
    mod = types.ModuleType("antenv.axon_hooks")
    mod.get_axon_ntff_profile_hook = lambda: hook
    mod.set_axon_ntff_profile_hook = lambda h: None
    sys.modules["antenv.axon_hooks"] = mod


def _build_program(CP, S, A, G):
    """Build the per-core Bass program. CP contexts/core, group size G."""
    f32 = mybir.dt.float32
    nc = bacc.Bacc("TRN2", target_bir_lowering=False)

    HS = S // 128  # 128-row K-chunks of the A contraction
    K = HS + 1     # matmuls per context (1 B-term + HS A-terms)
    assert S % 128 == 0 and A == 128
    NG = CP // CT
    CH = CT // 2   # contexts per half-group payload
    assert CP % CT == 0 and CT % 2 == 0
    WF = CH * K * S   # weight f32 per partition line per half-group
    AF = CH * K * G   # activation f32 per partition line per half-group

    # PSUM packing: FF contexts along the free dim of a bank, two 64-aligned
    # partition slots (matmul out base partition must be 0/32/64)
    FF = max(1, min(CT, 512 // S))
    PSL = 2 if G <= 64 else 1
    CPT = min(CT, PSL * FF)
    T = -(-CT // CPT)
    assert T * CPT == CT, (CT, FF, PSL, CPT)

    blob = nc.dram_tensor("blob", [NG, 2, 128, WF + AF], f32,
                          kind="ExternalInput").ap()
    adj_xx = nc.dram_tensor("adj_xx", [HS, 128, S], mybir.dt.uint8,
                            kind="ExternalInput").ap()
    adj_xu = nc.dram_tensor("adj_xu", [A, S], mybir.dt.uint8,
                            kind="ExternalInput").ap()
    # output blob: [group][partition-slot][sample][bank][context-half][s]
    out = nc.dram_tensor("out", [NG, PSL, G, T, FF, S], f32,
                         kind="ExternalOutput").ap()

    rounded = MM_DT == mybir.dt.float32r

    with tile.TileContext(nc) as tc:
        with (
            tc.tile_pool(name="const", bufs=1) as const,
            tc.tile_pool(name="w", bufs=W_BUFS) as wpool,
            tc.tile_pool(name="o", bufs=3) as opool,
            tc.tile_pool(name="psum", bufs=8, space="PSUM") as psum,
        ):
            # combined [adjB | adjA] mask: raw u8 over the fast HWDGE ring,
            # then one DVE cast-copy (also the same-engine funnel for the
            # mask TTs).  SWDGE cast-DMAs would cost ~17us of ramp-in.
            adjU = const.tile([128, K * S], mybir.dt.uint8)
            nc.sync.dma_start(adjU[:, :S], adj_xu[:])
            nc.sync.dma_start(
                adjU[:, S:].rearrange("p (h s) -> p h s", h=HS),
                adj_xx.rearrange("h p s -> p h s"))
            adjC = const.tile([128, K * S], f32)
            nc.vector.tensor_copy(adjC[:], adjU[:])
            adjC_b = adjC[:, None, :].to_broadcast([128, CH, K * S])

            for g in range(NG):
                halves = []
                for hf in range(2):
                    hb = wpool.tile([128, WF + AF], f32, tag="hb",
                                    name=f"hb_{g}_{hf}")
                    nc.sync.dma_start(hb[:], blob[g, hf])
                    wv = hb[:, :WF].rearrange("p (c k s) -> p c k s",
                                              c=CH, k=K)
                    av = hb[:, WF:].rearrange("p (c k g) -> p c k g",
                                              c=CH, k=K)
                    if rounded:
                        wm = wpool.tile([128, WF], MM_DT, tag="wm",
                                        name=f"wm_{g}_{hf}")
                        am = wpool.tile([128, AF], MM_DT, tag="am",
                                        name=f"am_{g}_{hf}")
                        nc.vector.tensor_copy(am[:], hb[:, WF:])
                        nc.vector.tensor_tensor(
                            wm[:].rearrange("p (c ks) -> p c ks", c=CH),
                            hb[:, :WF].rearrange("p (c ks) -> p c ks", c=CH),
                            adjC_b, mybir.AluOpType.mult)
                        wv = wm[:].rearrange("p (c k s) -> p c k s",
                                             c=CH, k=K)
                        av = am[:].rearrange("p (c k g) -> p c k g",
                                             c=CH, k=K)
                    else:
                        # mask B+A weights with ONE in-place multiply
                        nc.vector.tensor_tensor(
                            hb[:, :WF].rearrange("p (c ks) -> p c ks", c=CH),
                            hb[:, :WF].rearrange("p (c ks) -> p c ks", c=CH),
                            adjC_b, mybir.AluOpType.mult)
                    halves.append((wv, av))

                ps_tiles = [psum.tile([128, FF * S], f32, tag="ps",
                                      name=f"ps_{g}_{t}")
                            for t in range(T)]
                for c in range(CT):
                    hf, ci = divmod(c, CH)
                    wv, av = halves[hf]
                    t, r2 = divmod(c, CPT)
                    sl, cf = divmod(r2, FF)
                    pslice = ps_tiles[t][sl * 64:sl * 64 + G,
                                         cf * S:cf * S + S]
                    for k in range(K):
                        nc.tensor.matmul(
                            pslice,
                            lhsT=av[:, ci, k, :],
                            rhs=wv[:, ci, k, :],
                            start=(k == 0), stop=(k == K - 1))
                out_sb = opool.tile([128, T, FF, S], f32)
                for t in range(T):
                    for sl in range(PSL):
                        nc.scalar.copy(
                            out_sb[sl * 64:sl * 64 + G, t].rearrange(
                                "p f s -> p (f s)"),
                            ps_tiles[t][sl * 64:sl * 64 + G, :])
                for sl in range(PSL):
                    nc.scalar.dma_start(
                        out[g, sl], out_sb[sl * 64:sl * 64 + G])

    nc.compile()
    return nc


def kernel(x, u, WA, WB, adj_xx, adj_xu, context, _trace=False):
    B, S = x.shape
    _, A = u.shape
    C = WA.shape[0]
    assert C % N_CORES == 0
    CP = C // N_CORES
    HS = S // 128
    K = HS + 1
    NG = CP // CT
    CH = CT // 2

    # ---- host-side shard: group samples by context --------------------
    context = np.asarray(context)
    cnt = np.bincount(context, minlength=C)
    G = int(cnt.max())
    G = max(4, ((G + 3) // 4) * 4)
    order = np.argsort(context, kind="stable")
    starts = np.zeros(C + 1, np.int64)
    starts[1:] = np.cumsum(cnt)
    j = np.arange(G)
    valid = j[None, :] < cnt[:, None]                      # [C, G]
    pos = starts[:-1, None] + np.minimum(j[None, :],
                                         np.maximum(cnt[:, None] - 1, 0))
    gidx = order[pos]                                      # [C, G]

    Xp = np.asarray(x, np.float32)[gidx]                   # [C, G, S]
    Up = np.asarray(u, np.float32)[gidx]                   # [C, G, A]
    XpT = np.ascontiguousarray(Xp.transpose(0, 2, 1))      # [C, S, G]
    UpT = np.ascontiguousarray(Up.transpose(0, 2, 1))      # [C, A, G]

    WA = np.ascontiguousarray(WA, np.float32)
    WB = np.ascontiguousarray(WB, np.float32)
    adjxx_u8 = np.ascontiguousarray(adj_xx).view(np.uint8).reshape(HS, 128, S)
    adjxu_u8 = np.ascontiguousarray(adj_xu).view(np.uint8)

    WF = CH * K * S
    AF = CH * K * G
    in_maps = []
    for k in range(N_CORES):
        sl = slice(k * CP, (k + 1) * CP)
        # pack each half-group's weights+activations into one contiguous
        # blob: per partition line [CH, K, S] weights then [CH, K, G] acts,
        # slot 0 = B-term, slot 1+h = A-term K-chunk h
        W3 = np.empty((NG, 2, 128, CH, K, S), np.float32)
        W3[..., 0, :] = WB[sl].reshape(NG, 2, CH, 128, S) \
            .transpose(0, 1, 3, 2, 4)
        W3[..., 1:, :] = WA[sl].reshape(NG, 2, CH, HS, 128, S) \
            .transpose(0, 1, 4, 2, 3, 5)
        A3 = np.empty((NG, 2, 128, CH, K, G), np.float32)
        A3[..., 0, :] = UpT[sl].reshape(NG, 2, CH, 128, G) \
            .transpose(0, 1, 3, 2, 4)
        A3[..., 1:, :] = XpT[sl].reshape(NG, 2, CH, HS, 128, G) \
            .transpose(0, 1, 4, 2, 3, 5)
        blob = np.concatenate(
            [W3.reshape(NG, 2, 128, WF), A3.reshape(NG, 2, 128, AF)],
            axis=-1)
        in_maps.append({
            "blob": np.ascontiguousarray(blob),
            "adj_xx": adjxx_u8,
            "adj_xu": adjxu_u8,
        })

    if _trace:
        _install_profile_shim()
    nc = _build_program(CP, S, A, G)
    res = run_bass_kernel_spmd(nc, in_maps, core_ids=list(range(N_CORES)),
                               trace=_trace)

    # device output blobs [NG, PSL, G, T, FF, S] -> [CP, G, S].
    # context c in a group lives at bank t=c//CPT, partition slot
    # sl=(c%CPT)//FF (64-aligned), free half cf=c%FF.
    outs = []
    for r in res.results:
        v = r["out"]
        # axes (g, sl, gg, t, cf, s) -> (g, t, sl, cf, gg, s)
        v = v.transpose(0, 3, 1, 4, 2, 5).reshape(CP, G, S)
        outs.append(v)
    Out_all = np.concatenate(outs, axis=0)                 # [C, G, S]
    out_full = np.zeros((B, S), np.float32)
    out_full[gidx[valid]] = Out_all[valid]

    if _trace:
        return out_full, res
    return out_full
